# revision 16
# baseline (speedup 1.0000x reference)
"""Trainium2 Bass kernel for the Correlation module.

reference:
    affinities = einsum('lnd,ond->lon', x, upfold) / sqrt(d)   # [L,O,N]
    features   = einsum('lon,ond->lnd', sigmoid(affinities)-0.5, upfold)

Math: sigmoid(a)-0.5 = 0.5*tanh(a/2); with s = 1/sqrt(64) the weight is
W = tanh(A/16) (A = x @ upfold^T per n) and F = W @ (0.5*upfold), where
the 0.5 is folded into the host-prepared mm2 stationary.

Design (vs the v1 baseline at ~101.5us):

* Host-side layout prep. The kernel is data-parallel over N (8 n per
  core, processed as 4 pairs).  All transposes/casts move to numpy in
  kernel(): per pair the device receives XT=[128(n,d),1024 l],
  UT=[128(n,d),1024 o] (both bf16) and UB=[128 o%128, 8 oc,
  128(n,d)]*0.5 (bf16), and returns F^T=[128(n,d),1024 l] bf16 which the
  host transposes/casts back.  This removes every on-device DMA
  transpose, cast and staging copy of v1.

* The scalar-engine tanh chain was v1's ~79us bottleneck (64 ACTIVATE
  tiles x 1.23us; the Scalar engine is the only one with activation
  tables).  Half the tiles now go to the Vector engine via a CUSTOM DVE
  OP registered at import time: a degree-7 odd-polynomial Horner chain
  (((t+h)t+k)t+r)*y, y=g*a, t=y^2 -- exactly 8 ALU stages, one DVE
  instruction per [128,1024] tile (~1.24us, fp32 internal, bf16 out).
  The 4th coefficient r rides in via the documented C3->Src1 latch
  spill ([128,1] column).  GPSIMD (Pool) applies the +-1 clamp
  (tensor_scalar min/max, bf16 SBUF; Pool has no PSUM access so it
  cannot host any PSUM-reading stage).  Coefficients are a clamp-aware
  LP minimax fit (p>=1 on [2.2,3.3], p in [0.92,4] on [3.3,4.45],
  density^0.5-weighted bulk); end-to-end rel err 0.0055 if ALL tiles
  used the poly (gate 2e-2), 0.0050 at the 8/16 split used here.

* mm1/mm2 keep v1's PE packing: mm1 (A^T = U X^T, K=64) pairs the two
  n of a pair at tile_position (0,0)/(64,0); mm2 (F^T = (0.5u)^T W^T,
  K=128 over o-chunks) packs columns at (0,0)/(0,64).  PSUM: 3 'at'
  bufs + 1 'ft' buf = 8 banks.

Self-contained: hardcodes shapes + fitted coefficients.
"""

import numpy as np

L, N, D, O = 1024, 64, 64, 1024
NCORES = 8
NLOC = N // NCORES   # 8 n per core
NPAIRS = NLOC // 2   # 4 pairs

# deg-7 odd minimax fit of tanh on y in [-4.45, 4.45] (clamp-aware,
# y1=2.2, wexp=0.5): p(y) = b1 y + b3 y^3 + b5 y^5 + b7 y^7, then
# clamp(p, -1, 1).  y = a/16.
_B1, _B3, _B5, _B7 = 0.9750653, -0.2350052, 0.0318777, -0.0011094

# per-oc activation engine for (ni=0, ni=1):  'S' = scalar tanh,
# 'V' = custom-DVE poly + DVE clamp.  (GPSIMD tensor ops measured ~17us
# per [128,1024] tile on HW -- Q7 software loops -- so Pool is unusable.)
_PATTERN = ["SS", "SV", "SV", "SV", "SS", "SV", "SV", "SV"]

_CACHE = {}


def _poly_consts():
    """Map y-space coefficients to the custom-op constants.

    op computes (((t + h)*t + k)*t + r) * y with y = g*a, t = y*y, so
    p(a) = g^7 a^7 + h g^5 a^5 + k g^3 a^3 + r g a.  Match against
    b7' a^7 + b5' a^5 + b3' a^3 + b1' a  (bk' = bk/16^k).
    """
    b1, b3, b5, b7 = (_B1 / 16.0, _B3 / 16.0**3, _B5 / 16.0**5, _B7 / 16.0**7)
    g = float(np.sign(b7) * np.abs(b7) ** (1.0 / 7.0))
    h = float(b5 / g**5)
    k = float(b3 / g**3)
    r = float(b1 / g)
    return g, h, k, r


def _register_custom_op():
    """Register the deg-7 odd-Horner op in dve_ops.OPS (documented
    extension point; the repo is read-only so it is done in-process)."""
    import concourse.dve_ops as dve_ops_mod
    from concourse.dve_ops import DveOp
    from concourse.dve_spec import (
        Spec, Src0, C0, C1, C2, C3, _has_src1, _spill_c3_to_src1, lower, sq,
    )
    from concourse.dve_uop import DveOpSpec

    name = "ANT_ODD7_HORNER"
    for op in dve_ops_mod.OPS:
        if op.name == name:
            return op

    y = Src0 * C0
    t = sq(y)
    body = _spill_c3_to_src1((((t + C1) * t + C2) * t + C3) * y)

    def _ref(in0, in1, s0, s1, imm2):
        yy = in0.astype(np.float32) * s0
        tt = yy * yy
        rr = np.asarray(in1, np.float32).reshape(in1.shape[0], 1)
        return (((tt + s1) * tt + imm2) * tt + rr) * yy

    spec = Spec(body=body, reference=_ref)
    row = dve_ops_mod._CUSTOM_DVE_ROW_BASE + len(dve_ops_mod.OPS)
    assert row < 0x20
    ver = "v3"  # TRN2
    uops = lower(spec, ver=ver)
    sha = DveOpSpec(name=name, opcode=row, uops=uops,
                    rd1_en=_has_src1(spec)).sha(ver)
    op = DveOp(name, spec, subdim=False, uops_sha={ver: sha})
    dve_ops_mod.OPS.append(op)
    dve_ops_mod._SUB_OPCODE_FOR_NAME[name] = row
    dve_ops_mod.CUSTOM_DVE_SPECS[name] = spec
    return op


def _build_program():
    import concourse.mybir as mybir
    import concourse.tile as tile
    from concourse import bacc

    f32 = mybir.dt.float32
    bf16 = mybir.dt.bfloat16
    TANH = mybir.ActivationFunctionType.Tanh
    MAX = mybir.AluOpType.max
    MIN = mybir.AluOpType.min

    tanh7 = _register_custom_op()
    g_c, h_c, k_c, r_c = _poly_consts()

    nc = bacc.Bacc(
        "TRN2", target_bir_lowering=False, debug=False, num_devices=NCORES
    )
    xt_ap = nc.dram_tensor("xt", [NPAIRS, 128, 1024], bf16, kind="ExternalInput").ap()
    ut_ap = nc.dram_tensor("ut", [NPAIRS, 128, 1024], bf16, kind="ExternalInput").ap()
    ub_ap = nc.dram_tensor("ub", [NPAIRS, 128, 8, 128], bf16, kind="ExternalInput").ap()
    fo_ap = nc.dram_tensor("fo", [NPAIRS, 128, 1024], bf16, kind="ExternalOutput").ap()

    with tile.TileContext(nc) as tc:
        with (
            tc.tile_pool(name="inp", bufs=2) as inp,
            tc.tile_pool(name="wsb", bufs=3) as wsb,
            tc.tile_pool(name="aux", bufs=2) as aux,
            tc.tile_pool(name="atps", bufs=3, space="PSUM") as atps,
            tc.tile_pool(name="ftps", bufs=1, space="PSUM") as ftps,
        ):
            # [128,1] latched 4th coefficient for the custom op.
            rcol = aux.tile([128, 1], f32, tag="rcol", bufs=1)
            nc.vector.memset(rcol[:], r_c)
            # Warm the scalar engine's Tanh table during the initial DMA
            # window (ACT_TABLE_LOAD is ~1.3us and otherwise lands on the
            # first real activation's critical path).
            warm = aux.tile([128, 1], bf16, tag="warm", bufs=1)
            nc.scalar.activation(warm[:], rcol[:], TANH)

            loaded = {}

            def load_pair(p):
                XT = inp.tile([128, 1024], bf16, tag="XT")
                UT = inp.tile([128, 1024], bf16, tag="UT")
                UB = inp.tile([128, 8, 128], bf16, tag="UB")
                if p == 0:
                    # Startup latency: UB rides the idle gpsimd issue queue so
                    # XT/UT (which gate mm1) issue back-to-back on SP.
                    nc.sync.dma_start(XT[:], xt_ap[p])
                    nc.sync.dma_start(UT[:], ut_ap[p])
                    nc.gpsimd.dma_start(UB[:], ub_ap[p])
                else:
                    nc.sync.dma_start(XT[:], xt_ap[p])
                    nc.sync.dma_start(UT[:], ut_ap[p])
                    nc.sync.dma_start(UB[:], ub_ap[p])
                loaded[p] = (XT, UT, UB)

            def emit_out(p, fsb):
                nc.sync.dma_start(fo_ap[p], fsb[:])

            def act_scalar(at, ni):
                w = wsb.tile([128, 1024], bf16, tag=f"ws{ni}")
                nc.scalar.activation(w[:], at[:], TANH, scale=1.0 / 16.0)
                return w

            def act_poly(at, ni):
                wv = wsb.tile([128, 1024], bf16, tag=f"wv{ni}")
                nc.vector._custom_dve(
                    tanh7, out=wv[:], in0=at[:], in1=rcol[:],
                    s0=g_c, s1=h_c, imm2=k_c,
                )
                wc = wsb.tile([128, 1024], bf16, tag=f"wc{ni}")
                nc.vector.tensor_scalar(wc[:], wv[:], -1.0, 1.0, MAX, MIN)
                return wc

            def oc_loop(p, carry):
                XT, UT, UB = loaded.pop(p)
                ft = ftps.tile([128, 1024], f32, tag="ft")

                def mm1_pair(oc, at0, at1):
                    # ni-interleaved: adjacent queue entries hit disjoint PE
                    # row bands, so both lh steps run as concurrent pairs.
                    for lh in range(2):
                        for ni, at in ((0, at0), (1, at1)):
                            rows = slice(64 * ni, 64 * (ni + 1))
                            nc.tensor.matmul(
                                at[:, 512 * lh : 512 * (lh + 1)],
                                UT[rows, 128 * oc : 128 * (oc + 1)],
                                XT[rows, 512 * lh : 512 * (lh + 1)],
                                start=True,
                                stop=True,
                                tile_position=(64 * ni, 0),
                            )

                def mm2_one(oc, ni, lh, w):
                    rows = slice(64 * ni, 64 * (ni + 1))
                    nc.tensor.matmul(
                        ft[rows, 512 * lh : 512 * (lh + 1)],
                        UB[:, oc, rows],
                        w[:, 512 * lh : 512 * (lh + 1)],
                        start=(oc == 0),
                        stop=(oc == 7),
                        tile_position=(0, 64 * ni),
                    )

                def mm2_lh(oc, lh, w0, w1):
                    # lh-major, ni-interleaved: the two ni col-bands run as a
                    # concurrent pair, and each l-half of ft completes as a
                    # unit so the fsb half-casts can chase it.
                    for ni, w in ((0, w0), (1, w1)):
                        mm2_one(oc, ni, lh, w)

                prev = None  # (oc, w0, w1) awaiting mm2
                pending = None
                for oc in range(8):
                    if oc == 1 and carry is not None:
                        pending = carry["fsb"]()
                    if oc == 3 and pending is not None:
                        emit_out(*pending)
                    if oc == 0 and p + 1 < NPAIRS and p + 1 not in loaded:
                        load_pair(p + 1)
                    at0 = atps.tile([128, 1024], f32, tag="at")
                    at1 = atps.tile([128, 1024], f32, tag="at")
                    # Row-banded mm1 halves overlap on the PE (disjoint PE
                    # rows); col-banded mm2 halves of the previous oc follow.
                    mm1_pair(oc, at0, at1)
                    if prev is not None:
                        mm2_lh(prev[0], 0, prev[1], prev[2])
                        mm2_lh(prev[0], 1, prev[1], prev[2])
                    elif oc == 0 and carry is not None:
                        carry["mm2a"]()
                        carry["mm2b"]()
                    eng = _PATTERN[oc]
                    w0 = (act_scalar if eng[0] == "S" else act_poly)(at0, 0)
                    w1 = (act_scalar if eng[1] == "S" else act_poly)(at1, 1)
                    prev = (oc, w0, w1)

                def make_fsb():
                    # Two half-casts so ft's first bank pair frees as soon as
                    # the lh0 matmuls land (softens the ftps bufs=1 stall at
                    # the next pair's mm2 start).
                    fsb = aux.tile([128, 1024], bf16, tag="fsb")
                    nc.vector.tensor_copy(fsb[:, 0:512], ft[:, 0:512])
                    nc.vector.tensor_copy(fsb[:, 512:1024], ft[:, 512:1024])
                    return (p, fsb)

                return {
                    "mm2a": lambda: mm2_lh(7, 0, prev[1], prev[2]),
                    "mm2b": lambda: mm2_lh(7, 1, prev[1], prev[2]),
                    "fsb": make_fsb,
                }

            load_pair(0)
            carry = None
            for p in range(NPAIRS):
                carry = oc_loop(p, carry)
            carry["mm2a"]()
            carry["mm2b"]()
            emit_out(*carry["fsb"]())

    nc.compile()
    return nc


def _get_program():
    if "nc" not in _CACHE:
        _CACHE["nc"] = _build_program()
    return _CACHE["nc"]


def _make_in_maps(x, upfold):
    import ml_dtypes

    bf = ml_dtypes.bfloat16
    x = np.asarray(x, dtype=np.float32)
    upfold = np.asarray(upfold, dtype=np.float32)
    in_maps = []
    for c in range(NCORES):
        s = slice(NLOC * c, NLOC * (c + 1))
        xc = x[:, s, :]          # [L, 8, D]
        uc = upfold[:, s, :]     # [O, 8, D]
        xt = np.empty((NPAIRS, 128, 1024), bf)
        ut = np.empty((NPAIRS, 128, 1024), bf)
        ub = np.empty((NPAIRS, 128, 8, 128), bf)
        for p in range(NPAIRS):
            xp = xc[:, 2 * p : 2 * p + 2, :]   # [1024, 2, 64]
            up = uc[:, 2 * p : 2 * p + 2, :]   # [1024, 2, 64]
            xt[p] = xp.transpose(1, 2, 0).reshape(128, 1024).astype(bf)
            ut[p] = up.transpose(1, 2, 0).reshape(128, 1024).astype(bf)
            ub[p] = (
                (up * 0.5)
                .reshape(8, 128, 2, 64)
                .transpose(1, 0, 2, 3)
                .reshape(128, 8, 128)
                .astype(bf)
            )
        in_maps.append({"xt": xt, "ut": ut, "ub": ub})
    return in_maps


def _gather_out(results):
    out = np.empty((L, N, D), np.float32)
    for c in range(NCORES):
        fo = np.asarray(results[c]["fo"], dtype=np.float32)  # [4, 128, 1024]
        for p in range(NPAIRS):
            # fo[p][(nn*64+d), l] -> out[l, c*8 + 2p + nn, d]
            blk = fo[p].reshape(2, 64, 1024).transpose(2, 0, 1)  # [l, nn, d]
            out[:, NLOC * c + 2 * p : NLOC * c + 2 * p + 2, :] = blk
    return out


def run_sharded(x, upfold, trace=False, **kwargs):
    """Run on all 8 cores; returns (full_output, BassKernelResults)."""
    from concourse.bass_utils import run_bass_kernel_spmd

    nc = _get_program()
    res = run_bass_kernel_spmd(
        nc, _make_in_maps(x, upfold), core_ids=list(range(NCORES)),
        trace=trace, **kwargs
    )
    out = _gather_out(res.results)
    return out, res


def kernel(x, upfold):
    out, _ = run_sharded(x, upfold)
    return out


# revision 19
# speedup vs baseline: 1.1957x; 1.1957x over previous
"""Trainium2 Bass kernel for the Correlation module.

reference:
    affinities = einsum('lnd,ond->lon', x, upfold) / sqrt(d)   # [L,O,N]
    features   = einsum('lon,ond->lnd', sigmoid(affinities)-0.5, upfold)

Math: sigmoid(a)-0.5 = 0.5*tanh(a/2); with s = 1/sqrt(64) the weight is
W = tanh(A/16) (A = x @ upfold^T per n) and F = W @ (0.5*upfold), where
the 0.5 is folded into the host-prepared mm2 stationary.

Design (vs the v1 baseline at ~101.5us):

* Host-side layout prep. The kernel is data-parallel over N (8 n per
  core, processed as 4 pairs).  All transposes/casts move to numpy in
  kernel(): per pair the device receives XT=[128(n,d),1024 l],
  UT=[128(n,d),1024 o] (both bf16) and UB=[128 o%128, 8 oc,
  128(n,d)]*0.5 (bf16), and returns F^T=[128(n,d),1024 l] bf16 which the
  host transposes/casts back.  This removes every on-device DMA
  transpose, cast and staging copy of v1.

* The scalar-engine tanh chain was v1's ~79us bottleneck (64 ACTIVATE
  tiles x 1.23us; the Scalar engine is the only one with activation
  tables).  Half the tiles now go to the Vector engine via a CUSTOM DVE
  OP registered at import time: a degree-7 odd-polynomial Horner chain
  (((t+h)t+k)t+r)*y, y=g*a, t=y^2 -- exactly 8 ALU stages, one DVE
  instruction per [128,1024] tile (~1.24us, fp32 internal, bf16 out).
  The 4th coefficient r rides in via the documented C3->Src1 latch
  spill ([128,1] column).  GPSIMD (Pool) applies the +-1 clamp
  (tensor_scalar min/max, bf16 SBUF; Pool has no PSUM access so it
  cannot host any PSUM-reading stage).  Coefficients are a clamp-aware
  LP minimax fit (p>=1 on [2.2,3.3], p in [0.92,4] on [3.3,4.45],
  density^0.5-weighted bulk); end-to-end rel err 0.0055 if ALL tiles
  used the poly (gate 2e-2), 0.0050 at the 8/16 split used here.

* mm1/mm2 keep v1's PE packing: mm1 (A^T = U X^T, K=64) pairs the two
  n of a pair at tile_position (0,0)/(64,0); mm2 (F^T = (0.5u)^T W^T,
  K=128 over o-chunks) packs columns at (0,0)/(0,64).  PSUM: 3 'at'
  bufs + 1 'ft' buf = 8 banks.

Self-contained: hardcodes shapes + fitted coefficients.
"""

import numpy as np

L, N, D, O = 1024, 64, 64, 1024
NCORES = 8
NLOC = N // NCORES   # 8 n per core
NPAIRS = NLOC // 2   # 4 pairs

# deg-7 odd minimax fit of tanh on y in [-4.45, 4.45] (clamp-aware,
# y1=2.2, wexp=0.5): p(y) = b1 y + b3 y^3 + b5 y^5 + b7 y^7, then
# clamp(p, -1, 1).  y = a/16.
_B1, _B3, _B5, _B7 = 0.9750653, -0.2350052, 0.0318777, -0.0011094

# per-oc activation engine for (ni=0, ni=1):  'S' = scalar tanh,
# 'V' = custom-DVE poly + DVE clamp.  (GPSIMD tensor ops measured ~17us
# per [128,1024] tile on HW -- Q7 software loops -- so Pool is unusable.)
_PATTERN = ["SS", "SV", "SV", "SV", "SS", "SV", "SV", "SV"]

_CACHE = {}


def _poly_consts():
    """Map y-space coefficients to the custom-op constants.

    op computes (((t + h)*t + k)*t + r) * y with y = g*a, t = y*y, so
    p(a) = g^7 a^7 + h g^5 a^5 + k g^3 a^3 + r g a.  Match against
    b7' a^7 + b5' a^5 + b3' a^3 + b1' a  (bk' = bk/16^k).
    """
    b1, b3, b5, b7 = (_B1 / 16.0, _B3 / 16.0**3, _B5 / 16.0**5, _B7 / 16.0**7)
    g = float(np.sign(b7) * np.abs(b7) ** (1.0 / 7.0))
    h = float(b5 / g**5)
    k = float(b3 / g**3)
    r = float(b1 / g)
    return g, h, k, r


def _register_custom_op():
    """Register the deg-7 odd-Horner op in dve_ops.OPS (documented
    extension point; the repo is read-only so it is done in-process)."""
    import concourse.dve_ops as dve_ops_mod
    from concourse.dve_ops import DveOp
    from concourse.dve_spec import (
        Spec, Src0, C0, C1, C2, C3, _has_src1, _spill_c3_to_src1, lower, sq,
    )
    from concourse.dve_uop import DveOpSpec

    name = "ANT_ODD7_HORNER"
    for op in dve_ops_mod.OPS:
        if op.name == name:
            return op

    y = Src0 * C0
    t = sq(y)
    body = _spill_c3_to_src1((((t + C1) * t + C2) * t + C3) * y)

    def _ref(in0, in1, s0, s1, imm2):
        yy = in0.astype(np.float32) * s0
        tt = yy * yy
        rr = np.asarray(in1, np.float32).reshape(in1.shape[0], 1)
        return (((tt + s1) * tt + imm2) * tt + rr) * yy

    spec = Spec(body=body, reference=_ref)
    row = dve_ops_mod._CUSTOM_DVE_ROW_BASE + len(dve_ops_mod.OPS)
    assert row < 0x20
    ver = "v3"  # TRN2
    uops = lower(spec, ver=ver)
    sha = DveOpSpec(name=name, opcode=row, uops=uops,
                    rd1_en=_has_src1(spec)).sha(ver)
    op = DveOp(name, spec, subdim=False, uops_sha={ver: sha})
    dve_ops_mod.OPS.append(op)
    dve_ops_mod._SUB_OPCODE_FOR_NAME[name] = row
    dve_ops_mod.CUSTOM_DVE_SPECS[name] = spec
    return op


def _build_program():
    import concourse.mybir as mybir
    import concourse.tile as tile
    from concourse import bacc

    f32 = mybir.dt.float32
    bf16 = mybir.dt.bfloat16
    TANH = mybir.ActivationFunctionType.Tanh
    MAX = mybir.AluOpType.max
    MIN = mybir.AluOpType.min

    tanh7 = _register_custom_op()
    g_c, h_c, k_c, r_c = _poly_consts()

    nc = bacc.Bacc(
        "TRN2", target_bir_lowering=False, debug=False, num_devices=NCORES
    )
    xt_ap = nc.dram_tensor("xt", [NPAIRS, 128, 1024], bf16, kind="ExternalInput").ap()
    ut_ap = nc.dram_tensor("ut", [NPAIRS, 128, 1024], bf16, kind="ExternalInput").ap()
    ub_ap = nc.dram_tensor("ub", [NPAIRS, 128, 8, 128], bf16, kind="ExternalInput").ap()
    fo_ap = nc.dram_tensor("fo", [NPAIRS, 128, 1024], bf16, kind="ExternalOutput").ap()

    with tile.TileContext(nc) as tc:
        with (
            tc.tile_pool(name="inp", bufs=2) as inp,
            tc.tile_pool(name="wsb", bufs=3) as wsb,
            tc.tile_pool(name="aux", bufs=2) as aux,
            tc.tile_pool(name="atps", bufs=3, space="PSUM") as atps,
            tc.tile_pool(name="ftps", bufs=1, space="PSUM") as ftps,
        ):
            # [128,1] latched 4th coefficient for the custom op.
            rcol = aux.tile([128, 1], f32, tag="rcol", bufs=1)
            nc.vector.memset(rcol[:], r_c)
            # Warm the scalar engine's Tanh table during the initial DMA
            # window (ACT_TABLE_LOAD is ~1.3us and otherwise lands on the
            # first real activation's critical path).
            warm = aux.tile([128, 1], bf16, tag="warm", bufs=1)
            nc.scalar.activation(warm[:], rcol[:], TANH)

            loaded = {}

            def load_pair(p):
                XT = inp.tile([128, 1024], bf16, tag="XT")
                UT = inp.tile([128, 1024], bf16, tag="UT")
                UB = inp.tile([128, 8, 128], bf16, tag="UB")
                if p == 0:
                    # Startup latency: UB rides the idle gpsimd issue queue so
                    # XT/UT (which gate mm1) issue back-to-back on SP.
                    nc.sync.dma_start(XT[:], xt_ap[p])
                    nc.sync.dma_start(UT[:], ut_ap[p])
                    nc.gpsimd.dma_start(UB[:], ub_ap[p])
                else:
                    nc.sync.dma_start(XT[:], xt_ap[p])
                    nc.sync.dma_start(UT[:], ut_ap[p])
                    nc.sync.dma_start(UB[:], ub_ap[p])
                loaded[p] = (XT, UT, UB)

            def emit_out(p, fsb):
                nc.sync.dma_start(fo_ap[p], fsb[:])

            def act_scalar(at, ni):
                w = wsb.tile([128, 1024], bf16, tag=f"ws{ni}")
                nc.scalar.activation(w[:], at[:], TANH, scale=1.0 / 16.0)
                return w

            def act_poly(at, ni):
                wv = wsb.tile([128, 1024], bf16, tag=f"wv{ni}")
                nc.vector._custom_dve(
                    tanh7, out=wv[:], in0=at[:], in1=rcol[:],
                    s0=g_c, s1=h_c, imm2=k_c,
                )
                wc = wsb.tile([128, 1024], bf16, tag=f"wc{ni}")
                nc.vector.tensor_scalar(wc[:], wv[:], -1.0, 1.0, MAX, MIN)
                return wc

            def oc_loop(p, carry):
                XT, UT, UB = loaded.pop(p)
                ft = ftps.tile([128, 1024], f32, tag="ft")

                def mm1_pair(oc, at0, at1):
                    # ni-major: the PE streams ~2 bf16 cols/cycle regardless
                    # of tile_position pairing (measured), and this order
                    # meets the at-ring/activation dependency waits latest.
                    for ni, at in ((0, at0), (1, at1)):
                        rows = slice(64 * ni, 64 * (ni + 1))
                        for lh in range(2):
                            nc.tensor.matmul(
                                at[:, 512 * lh : 512 * (lh + 1)],
                                UT[rows, 128 * oc : 128 * (oc + 1)],
                                XT[rows, 512 * lh : 512 * (lh + 1)],
                                start=True,
                                stop=True,
                                tile_position=(64 * ni, 0),
                            )

                def mm2_one(oc, ni, lh, w):
                    rows = slice(64 * ni, 64 * (ni + 1))
                    nc.tensor.matmul(
                        ft[rows, 512 * lh : 512 * (lh + 1)],
                        UB[:, oc, rows],
                        w[:, 512 * lh : 512 * (lh + 1)],
                        start=(oc == 0),
                        stop=(oc == 7),
                        tile_position=(0, 64 * ni),
                    )

                def mm2_half(oc, ni, w):
                    # ni-major in the steady loop: the first two matmuls
                    # depend only on w0 (the faster scalar tile), so the PE
                    # reaches the slower poly tile's dependency later.
                    for lh in range(2):
                        mm2_one(oc, ni, lh, w)

                def mm2_lh(oc, lh, w0, w1):
                    # lh-major at the pair boundary (both w long since done):
                    # each l-half of ft completes as a unit so the fsb
                    # half-casts can chase it.
                    for ni, w in ((0, w0), (1, w1)):
                        mm2_one(oc, ni, lh, w)

                prev = None  # (oc, w0, w1) awaiting mm2
                pending = None
                for oc in range(8):
                    if oc == 1 and carry is not None:
                        pending = carry["fsb"]()
                    if oc == 3 and pending is not None:
                        emit_out(*pending)
                    if oc == 0 and p + 1 < NPAIRS and p + 1 not in loaded:
                        load_pair(p + 1)
                    at0 = atps.tile([128, 1024], f32, tag="at")
                    at1 = atps.tile([128, 1024], f32, tag="at")
                    # Row-banded mm1 halves overlap on the PE (disjoint PE
                    # rows); col-banded mm2 halves of the previous oc follow.
                    mm1_pair(oc, at0, at1)
                    if prev is not None:
                        mm2_half(prev[0], 0, prev[1])
                        mm2_half(prev[0], 1, prev[2])
                    elif oc == 0 and carry is not None:
                        carry["mm2a"]()
                        carry["mm2b"]()
                    eng = _PATTERN[oc]
                    w0 = (act_scalar if eng[0] == "S" else act_poly)(at0, 0)
                    w1 = (act_scalar if eng[1] == "S" else act_poly)(at1, 1)
                    prev = (oc, w0, w1)

                def make_fsb():
                    # Two half-casts so ft's first bank pair frees as soon as
                    # the lh0 matmuls land (softens the ftps bufs=1 stall at
                    # the next pair's mm2 start).
                    fsb = aux.tile([128, 1024], bf16, tag="fsb")
                    nc.vector.tensor_copy(fsb[:, 0:512], ft[:, 0:512])
                    nc.vector.tensor_copy(fsb[:, 512:1024], ft[:, 512:1024])
                    return (p, fsb)

                return {
                    "mm2a": lambda: mm2_lh(7, 0, prev[1], prev[2]),
                    "mm2b": lambda: mm2_lh(7, 1, prev[1], prev[2]),
                    "fsb": make_fsb,
                }

            load_pair(0)
            carry = None
            for p in range(NPAIRS):
                carry = oc_loop(p, carry)
            carry["mm2a"]()
            carry["mm2b"]()
            emit_out(*carry["fsb"]())

    nc.compile()
    return nc


def _get_program():
    if "nc" not in _CACHE:
        _CACHE["nc"] = _build_program()
    return _CACHE["nc"]


def _make_in_maps(x, upfold):
    import ml_dtypes

    bf = ml_dtypes.bfloat16
    x = np.asarray(x, dtype=np.float32)
    upfold = np.asarray(upfold, dtype=np.float32)
    in_maps = []
    for c in range(NCORES):
        s = slice(NLOC * c, NLOC * (c + 1))
        xc = x[:, s, :]          # [L, 8, D]
        uc = upfold[:, s, :]     # [O, 8, D]
        xt = np.empty((NPAIRS, 128, 1024), bf)
        ut = np.empty((NPAIRS, 128, 1024), bf)
        ub = np.empty((NPAIRS, 128, 8, 128), bf)
        for p in range(NPAIRS):
            xp = xc[:, 2 * p : 2 * p + 2, :]   # [1024, 2, 64]
            up = uc[:, 2 * p : 2 * p + 2, :]   # [1024, 2, 64]
            xt[p] = xp.transpose(1, 2, 0).reshape(128, 1024).astype(bf)
            ut[p] = up.transpose(1, 2, 0).reshape(128, 1024).astype(bf)
            ub[p] = (
                (up * 0.5)
                .reshape(8, 128, 2, 64)
                .transpose(1, 0, 2, 3)
                .reshape(128, 8, 128)
                .astype(bf)
            )
        in_maps.append({"xt": xt, "ut": ut, "ub": ub})
    return in_maps


def _gather_out(results):
    out = np.empty((L, N, D), np.float32)
    for c in range(NCORES):
        fo = np.asarray(results[c]["fo"], dtype=np.float32)  # [4, 128, 1024]
        for p in range(NPAIRS):
            # fo[p][(nn*64+d), l] -> out[l, c*8 + 2p + nn, d]
            blk = fo[p].reshape(2, 64, 1024).transpose(2, 0, 1)  # [l, nn, d]
            out[:, NLOC * c + 2 * p : NLOC * c + 2 * p + 2, :] = blk
    return out


def run_sharded(x, upfold, trace=False, **kwargs):
    """Run on all 8 cores; returns (full_output, BassKernelResults)."""
    from concourse.bass_utils import run_bass_kernel_spmd

    nc = _get_program()
    res = run_bass_kernel_spmd(
        nc, _make_in_maps(x, upfold), core_ids=list(range(NCORES)),
        trace=trace, **kwargs
    )
    out = _gather_out(res.results)
    return out, res


def kernel(x, upfold):
    out, _ = run_sharded(x, upfold)
    return out


# revision 20
# speedup vs baseline: 1.2327x; 1.0310x over previous
"""Trainium2 Bass kernel for the Correlation module.

reference:
    affinities = einsum('lnd,ond->lon', x, upfold) / sqrt(d)   # [L,O,N]
    features   = einsum('lon,ond->lnd', sigmoid(affinities)-0.5, upfold)

Math: sigmoid(a)-0.5 = 0.5*tanh(a/2); with s = 1/sqrt(64) the weight is
W = tanh(A/16) (A = x @ upfold^T per n) and F = W @ (0.5*upfold), where
the 0.5 is folded into the host-prepared mm2 stationary.

Design (vs the v1 baseline at ~101.5us):

* Host-side layout prep. The kernel is data-parallel over N (8 n per
  core, processed as 4 pairs).  All transposes/casts move to numpy in
  kernel(): per pair the device receives XT=[128(n,d),1024 l],
  UT=[128(n,d),1024 o] (both bf16) and UB=[128 o%128, 8 oc,
  128(n,d)]*0.5 (bf16), and returns F^T=[128(n,d),1024 l] bf16 which the
  host transposes/casts back.  This removes every on-device DMA
  transpose, cast and staging copy of v1.

* The scalar-engine tanh chain was v1's ~79us bottleneck (64 ACTIVATE
  tiles x 1.23us; the Scalar engine is the only one with activation
  tables).  Half the tiles now go to the Vector engine via a CUSTOM DVE
  OP registered at import time: a degree-7 odd-polynomial Horner chain
  (((t+h)t+k)t+r)*y, y=g*a, t=y^2 -- exactly 8 ALU stages, one DVE
  instruction per [128,1024] tile (~1.24us, fp32 internal, bf16 out).
  The 4th coefficient r rides in via the documented C3->Src1 latch
  spill ([128,1] column).  GPSIMD (Pool) applies the +-1 clamp
  (tensor_scalar min/max, bf16 SBUF; Pool has no PSUM access so it
  cannot host any PSUM-reading stage).  Coefficients are a clamp-aware
  LP minimax fit (p>=1 on [2.2,3.3], p in [0.92,4] on [3.3,4.45],
  density^0.5-weighted bulk); end-to-end rel err 0.0055 if ALL tiles
  used the poly (gate 2e-2), 0.0050 at the 8/16 split used here.

* mm1/mm2 keep v1's PE packing: mm1 (A^T = U X^T, K=64) pairs the two
  n of a pair at tile_position (0,0)/(64,0); mm2 (F^T = (0.5u)^T W^T,
  K=128 over o-chunks) packs columns at (0,0)/(0,64).  PSUM: 3 'at'
  bufs + 1 'ft' buf = 8 banks.

Self-contained: hardcodes shapes + fitted coefficients.
"""

import numpy as np

L, N, D, O = 1024, 64, 64, 1024
NCORES = 8
NLOC = N // NCORES   # 8 n per core
NPAIRS = NLOC // 2   # 4 pairs

# deg-7 odd minimax fit of tanh on y in [-4.45, 4.45] (clamp-aware,
# y1=2.2, wexp=0.5): p(y) = b1 y + b3 y^3 + b5 y^5 + b7 y^7, then
# clamp(p, -1, 1).  y = a/16.
_B1, _B3, _B5, _B7 = 0.9750653, -0.2350052, 0.0318777, -0.0011094

# per-oc activation engine for (ni=0, ni=1):  'S' = scalar tanh,
# 'V' = custom-DVE poly + DVE clamp.  (GPSIMD tensor ops measured ~17us
# per [128,1024] tile on HW -- Q7 software loops -- so Pool is unusable.)
_PATTERN = ["SS", "SV", "SV", "SV", "SV", "SV", "SV", "SV"]

_CACHE = {}


def _poly_consts():
    """Map y-space coefficients to the custom-op constants.

    op computes (((t + h)*t + k)*t + r) * y with y = g*a, t = y*y, so
    p(a) = g^7 a^7 + h g^5 a^5 + k g^3 a^3 + r g a.  Match against
    b7' a^7 + b5' a^5 + b3' a^3 + b1' a  (bk' = bk/16^k).
    """
    b1, b3, b5, b7 = (_B1 / 16.0, _B3 / 16.0**3, _B5 / 16.0**5, _B7 / 16.0**7)
    g = float(np.sign(b7) * np.abs(b7) ** (1.0 / 7.0))
    h = float(b5 / g**5)
    k = float(b3 / g**3)
    r = float(b1 / g)
    return g, h, k, r


def _register_custom_op():
    """Register the deg-7 odd-Horner op in dve_ops.OPS (documented
    extension point; the repo is read-only so it is done in-process)."""
    import concourse.dve_ops as dve_ops_mod
    from concourse.dve_ops import DveOp
    from concourse.dve_spec import (
        Spec, Src0, C0, C1, C2, C3, _has_src1, _spill_c3_to_src1, lower, sq,
    )
    from concourse.dve_uop import DveOpSpec

    name = "ANT_ODD7_HORNER"
    for op in dve_ops_mod.OPS:
        if op.name == name:
            return op

    y = Src0 * C0
    t = sq(y)
    body = _spill_c3_to_src1((((t + C1) * t + C2) * t + C3) * y)

    def _ref(in0, in1, s0, s1, imm2):
        yy = in0.astype(np.float32) * s0
        tt = yy * yy
        rr = np.asarray(in1, np.float32).reshape(in1.shape[0], 1)
        return (((tt + s1) * tt + imm2) * tt + rr) * yy

    spec = Spec(body=body, reference=_ref)
    row = dve_ops_mod._CUSTOM_DVE_ROW_BASE + len(dve_ops_mod.OPS)
    assert row < 0x20
    ver = "v3"  # TRN2
    uops = lower(spec, ver=ver)
    sha = DveOpSpec(name=name, opcode=row, uops=uops,
                    rd1_en=_has_src1(spec)).sha(ver)
    op = DveOp(name, spec, subdim=False, uops_sha={ver: sha})
    dve_ops_mod.OPS.append(op)
    dve_ops_mod._SUB_OPCODE_FOR_NAME[name] = row
    dve_ops_mod.CUSTOM_DVE_SPECS[name] = spec
    return op


def _build_program():
    import concourse.mybir as mybir
    import concourse.tile as tile
    from concourse import bacc

    f32 = mybir.dt.float32
    bf16 = mybir.dt.bfloat16
    TANH = mybir.ActivationFunctionType.Tanh
    MAX = mybir.AluOpType.max
    MIN = mybir.AluOpType.min

    tanh7 = _register_custom_op()
    g_c, h_c, k_c, r_c = _poly_consts()

    nc = bacc.Bacc(
        "TRN2", target_bir_lowering=False, debug=False, num_devices=NCORES
    )
    xt_ap = nc.dram_tensor("xt", [NPAIRS, 128, 1024], bf16, kind="ExternalInput").ap()
    ut_ap = nc.dram_tensor("ut", [NPAIRS, 128, 1024], bf16, kind="ExternalInput").ap()
    ub_ap = nc.dram_tensor("ub", [NPAIRS, 128, 8, 128], bf16, kind="ExternalInput").ap()
    fo_ap = nc.dram_tensor("fo", [NPAIRS, 128, 1024], bf16, kind="ExternalOutput").ap()

    with tile.TileContext(nc) as tc:
        with (
            tc.tile_pool(name="inp", bufs=2) as inp,
            tc.tile_pool(name="wsb", bufs=3) as wsb,
            tc.tile_pool(name="aux", bufs=2) as aux,
            tc.tile_pool(name="atps", bufs=3, space="PSUM") as atps,
            tc.tile_pool(name="ftps", bufs=1, space="PSUM") as ftps,
        ):
            # [128,1] latched 4th coefficient for the custom op.
            rcol = aux.tile([128, 1], f32, tag="rcol", bufs=1)
            nc.vector.memset(rcol[:], r_c)
            # Warm the scalar engine's Tanh table during the initial DMA
            # window (ACT_TABLE_LOAD is ~1.3us and otherwise lands on the
            # first real activation's critical path).
            warm = aux.tile([128, 1], bf16, tag="warm", bufs=1)
            nc.scalar.activation(warm[:], rcol[:], TANH)

            loaded = {}

            def load_pair(p):
                XT = inp.tile([128, 1024], bf16, tag="XT")
                UT = inp.tile([128, 1024], bf16, tag="UT")
                UB = inp.tile([128, 8, 128], bf16, tag="UB")
                if p == 0:
                    # Startup latency: UB rides the idle gpsimd issue queue so
                    # XT/UT (which gate mm1) issue back-to-back on SP.
                    nc.sync.dma_start(XT[:], xt_ap[p])
                    nc.sync.dma_start(UT[:], ut_ap[p])
                    nc.gpsimd.dma_start(UB[:], ub_ap[p])
                else:
                    nc.sync.dma_start(XT[:], xt_ap[p])
                    nc.sync.dma_start(UT[:], ut_ap[p])
                    nc.sync.dma_start(UB[:], ub_ap[p])
                loaded[p] = (XT, UT, UB)

            def emit_out(p, fsb):
                nc.sync.dma_start(fo_ap[p], fsb[:])

            def act_scalar(at, ni):
                w = wsb.tile([128, 1024], bf16, tag=f"ws{ni}")
                nc.scalar.activation(w[:], at[:], TANH, scale=1.0 / 16.0)
                return w

            def act_poly(at, ni):
                wv = wsb.tile([128, 1024], bf16, tag=f"wv{ni}")
                nc.vector._custom_dve(
                    tanh7, out=wv[:], in0=at[:], in1=rcol[:],
                    s0=g_c, s1=h_c, imm2=k_c,
                )
                wc = wsb.tile([128, 1024], bf16, tag=f"wc{ni}")
                nc.vector.tensor_scalar(wc[:], wv[:], -1.0, 1.0, MAX, MIN)
                return wc

            def oc_loop(p, carry):
                XT, UT, UB = loaded.pop(p)
                ft = ftps.tile([128, 1024], f32, tag="ft")

                def mm1_pair(oc, at0, at1):
                    # ni-major: the PE streams ~2 bf16 cols/cycle regardless
                    # of tile_position pairing (measured), and this order
                    # meets the at-ring/activation dependency waits latest.
                    for ni, at in ((0, at0), (1, at1)):
                        rows = slice(64 * ni, 64 * (ni + 1))
                        for lh in range(2):
                            nc.tensor.matmul(
                                at[:, 512 * lh : 512 * (lh + 1)],
                                UT[rows, 128 * oc : 128 * (oc + 1)],
                                XT[rows, 512 * lh : 512 * (lh + 1)],
                                start=True,
                                stop=True,
                                tile_position=(64 * ni, 0),
                            )

                def mm2_one(oc, ni, lh, w):
                    rows = slice(64 * ni, 64 * (ni + 1))
                    nc.tensor.matmul(
                        ft[rows, 512 * lh : 512 * (lh + 1)],
                        UB[:, oc, rows],
                        w[:, 512 * lh : 512 * (lh + 1)],
                        start=(oc == 0),
                        stop=(oc == 7),
                        tile_position=(0, 64 * ni),
                    )

                def mm2_half(oc, ni, w):
                    # ni-major in the steady loop: the first two matmuls
                    # depend only on w0 (the faster scalar tile), so the PE
                    # reaches the slower poly tile's dependency later.
                    for lh in range(2):
                        mm2_one(oc, ni, lh, w)

                def mm2_lh(oc, lh, w0, w1):
                    # lh-major at the pair boundary (both w long since done):
                    # each l-half of ft completes as a unit so the fsb
                    # half-casts can chase it.
                    for ni, w in ((0, w0), (1, w1)):
                        mm2_one(oc, ni, lh, w)

                prev = None  # (oc, w0, w1) awaiting mm2
                pending = None
                for oc in range(8):
                    if oc == 1 and carry is not None:
                        pending = carry["fsb"]()
                    if oc == 3 and pending is not None:
                        emit_out(*pending)
                    if oc == 0 and p + 1 < NPAIRS and p + 1 not in loaded:
                        load_pair(p + 1)
                    at0 = atps.tile([128, 1024], f32, tag="at")
                    at1 = atps.tile([128, 1024], f32, tag="at")
                    # Row-banded mm1 halves overlap on the PE (disjoint PE
                    # rows); col-banded mm2 halves of the previous oc follow.
                    mm1_pair(oc, at0, at1)
                    if prev is not None:
                        mm2_half(prev[0], 0, prev[1])
                        mm2_half(prev[0], 1, prev[2])
                    elif oc == 0 and carry is not None:
                        carry["mm2a"]()
                        carry["mm2b"]()
                    eng = _PATTERN[oc]
                    w0 = (act_scalar if eng[0] == "S" else act_poly)(at0, 0)
                    w1 = (act_scalar if eng[1] == "S" else act_poly)(at1, 1)
                    prev = (oc, w0, w1)

                def make_fsb():
                    # Two half-casts so ft's first bank pair frees as soon as
                    # the lh0 matmuls land (softens the ftps bufs=1 stall at
                    # the next pair's mm2 start).
                    fsb = aux.tile([128, 1024], bf16, tag="fsb")
                    nc.vector.tensor_copy(fsb[:, 0:512], ft[:, 0:512])
                    nc.vector.tensor_copy(fsb[:, 512:1024], ft[:, 512:1024])
                    return (p, fsb)

                return {
                    "mm2a": lambda: mm2_lh(7, 0, prev[1], prev[2]),
                    "mm2b": lambda: mm2_lh(7, 1, prev[1], prev[2]),
                    "fsb": make_fsb,
                }

            load_pair(0)
            carry = None
            for p in range(NPAIRS):
                carry = oc_loop(p, carry)
            carry["mm2a"]()
            carry["mm2b"]()
            emit_out(*carry["fsb"]())

    nc.compile()
    return nc


def _get_program():
    if "nc" not in _CACHE:
        _CACHE["nc"] = _build_program()
    return _CACHE["nc"]


def _make_in_maps(x, upfold):
    import ml_dtypes

    bf = ml_dtypes.bfloat16
    x = np.asarray(x, dtype=np.float32)
    upfold = np.asarray(upfold, dtype=np.float32)
    in_maps = []
    for c in range(NCORES):
        s = slice(NLOC * c, NLOC * (c + 1))
        xc = x[:, s, :]          # [L, 8, D]
        uc = upfold[:, s, :]     # [O, 8, D]
        xt = np.empty((NPAIRS, 128, 1024), bf)
        ut = np.empty((NPAIRS, 128, 1024), bf)
        ub = np.empty((NPAIRS, 128, 8, 128), bf)
        for p in range(NPAIRS):
            xp = xc[:, 2 * p : 2 * p + 2, :]   # [1024, 2, 64]
            up = uc[:, 2 * p : 2 * p + 2, :]   # [1024, 2, 64]
            xt[p] = xp.transpose(1, 2, 0).reshape(128, 1024).astype(bf)
            ut[p] = up.transpose(1, 2, 0).reshape(128, 1024).astype(bf)
            ub[p] = (
                (up * 0.5)
                .reshape(8, 128, 2, 64)
                .transpose(1, 0, 2, 3)
                .reshape(128, 8, 128)
                .astype(bf)
            )
        in_maps.append({"xt": xt, "ut": ut, "ub": ub})
    return in_maps


def _gather_out(results):
    out = np.empty((L, N, D), np.float32)
    for c in range(NCORES):
        fo = np.asarray(results[c]["fo"], dtype=np.float32)  # [4, 128, 1024]
        for p in range(NPAIRS):
            # fo[p][(nn*64+d), l] -> out[l, c*8 + 2p + nn, d]
            blk = fo[p].reshape(2, 64, 1024).transpose(2, 0, 1)  # [l, nn, d]
            out[:, NLOC * c + 2 * p : NLOC * c + 2 * p + 2, :] = blk
    return out


def run_sharded(x, upfold, trace=False, **kwargs):
    """Run on all 8 cores; returns (full_output, BassKernelResults)."""
    from concourse.bass_utils import run_bass_kernel_spmd

    nc = _get_program()
    res = run_bass_kernel_spmd(
        nc, _make_in_maps(x, upfold), core_ids=list(range(NCORES)),
        trace=trace, **kwargs
    )
    out = _gather_out(res.results)
    return out, res


def kernel(x, upfold):
    out, _ = run_sharded(x, upfold)
    return out
